# revision 20
# baseline (speedup 1.0000x reference)
"""TRN2 Bass kernel for nn_BatchedCauchyKernel3d.

reference:
    d   = clip(||x_n||^2 + ||y_m||^2 - 2 x_n.y_m, 1e-10, 1e6)
    sxy = sqrt(clip(scale_x_n * scale_y_m, 1e-10, 1e12))
    out = 1 / (1 + d / sxy)

Rewrite: with u_n = sqrt(scale_x_n), v_m = sqrt(scale_y_m):
    1 + d/sxy = sum_k XA[k,n] * YA[k,m]      (K = 6 augmented contraction)
      XA = [-2 x1/u, -2 x2/u, -2 x3/u, ||x||^2/u, 1/u, 1]
      YA = [   y1/v,    y2/v,    y3/v,       1/v, ||y||^2/v, 1]
so the whole kernel matrix is ONE matmul followed by an elementwise
reciprocal.  The matmul runs in bf16 with a 3-way hi/mid/lo split of each
operand (6 cross-term pairs -> K = 36), which reproduces fp32 accuracy at
full (1 col/cycle) PE speed; fp32-native matmuls are 4x slower on TRN2.

Sharding: 8 cores, core c owns batch c//2, row half c%2 -> a (2048, 4096)
f32 output block per core (the output DMA of 32 MB/core is the roofline).
"""

import sys

if "/opt/trn_rl_repo" not in sys.path:
    sys.path.insert(0, "/opt/trn_rl_repo")

import numpy as np

B, NX, NY, FDIM = 4, 4096, 4096, 16
NCORES = 8
R = B * NX // NCORES  # 2048 rows per core
KPAIRS = 6  # (h,h),(h,m),(m,h),(h,l),(m,m),(l,h)
KR = 6 * KPAIRS  # 36

_CACHE = {}


def _build_program(rows, ny):
    from contextlib import ExitStack

    import concourse.tile as tile
    from concourse import bacc, mybir

    BF16 = mybir.dt.bfloat16
    F32 = mybir.dt.float32

    NB = 512  # matmul moving free dim (one PSUM bank of fp32)
    CH = 2048  # reciprocal chunk = 4 PSUM banks

    nc = bacc.Bacc("TRN2", target_bir_lowering=False, debug=False)
    xya = nc.declare_dram_parameter("xya", [KR, rows + ny], BF16, isOutput=False)
    out = nc.declare_dram_parameter("out", [rows, ny], F32, isOutput=True)

    with ExitStack() as ctx:
        tc = ctx.enter_context(tile.TileContext(nc))
        const = ctx.enter_context(tc.tile_pool(name="const", bufs=1))
        psum = ctx.enter_context(tc.tile_pool(name="psum", bufs=2, space="PSUM"))
        outp = ctx.enter_context(tc.tile_pool(name="outp", bufs=6))
        actt = ctx.enter_context(tc.tile_pool(name="actt", bufs=2))

        # Load the 36 contraction rows (split by column range across three
        # engines' DMA queues so the first matmuls only wait on the slices
        # they read), then duplicate them on-chip to partitions 64-99 so
        # matmuls can alternate PE row-groups and run concurrently in
        # disjoint quadrants of the array.
        xya_sb = const.tile([64 + KR, rows + ny], BF16)
        ranges = [(0, rows + NB), (rows + NB, rows + CH), (rows + CH, rows + ny)]
        engines = [nc.scalar, nc.sync, nc.gpsimd]
        for (lo, hi), eng in zip(ranges, engines):
            eng.dma_start(xya_sb[0:KR, lo:hi], xya[:, lo:hi])
            eng.dma_start(xya_sb[64 : 64 + KR, lo:hi], xya_sb[0:KR, lo:hi])

        for m in range(rows // 128):
            for h in range(ny // CH):
                ps = psum.tile([128, CH], F32, tag="ps")
                for j in range(CH // NB):
                    col = h * CH + j * NB
                    # first row-tile stays on group A: its matmuls gate the
                    # pipeline ramp and must not wait for the duplicate copy
                    g = 0 if m == 0 else 64 * (j % 2)
                    nc.tensor.matmul(
                        ps[:, j * NB : (j + 1) * NB],
                        xya_sb[g : g + KR, m * 128 : (m + 1) * 128],
                        xya_sb[g : g + KR, rows + col : rows + col + NB],
                        start=True,
                        stop=True,
                        tile_position=(g, 0),
                    )
                # fine-grained epilogue for the first row-tile so output DMA
                # starts as early as possible
                ot = outp.tile([128, CH], F32)
                k = 2 * m + h
                if m == 0:
                    for j in range(CH // NB):
                        sl = slice(j * NB, (j + 1) * NB)
                        nc.vector.reciprocal_approx_fast(out=ot[:, sl], in_=ps[:, sl])
                        nc.sync.dma_start(
                            out[0:128, h * CH + j * NB : h * CH + (j + 1) * NB],
                            ot[:, sl],
                        )
                elif k % 3 == 2:
                    # every 3rd chunk on the otherwise-idle ScalarEngine:
                    # 1/(1+t) = exp(-ln(1+t)); Ln and Exp share one ACT table
                    # set.  Takes ~1/3 of the chunks off the DVE, which
                    # otherwise caps chunk supply during the pipeline ramp.
                    lt = actt.tile([128, CH], F32)
                    nc.scalar.activation(lt[:], ps[:], mybir.ActivationFunctionType.Ln)
                    nc.scalar.activation(
                        ot[:], lt[:], mybir.ActivationFunctionType.Exp, scale=-1.0
                    )
                    nc.sync.dma_start(
                        out[m * 128 : (m + 1) * 128, h * CH : (h + 1) * CH], ot[:]
                    )
                else:
                    nc.vector.reciprocal_approx_fast(out=ot[:], in_=ps[:])
                    nc.sync.dma_start(
                        out[m * 128 : (m + 1) * 128, h * CH : (h + 1) * CH], ot[:]
                    )

    nc.compile()
    return nc


def _get_program(rows=R, ny=NY):
    key = (rows, ny)
    if key not in _CACHE:
        _CACHE[key] = _build_program(rows, ny)
    return _CACHE[key]


def _augment(x, y, sample_x, sample_y, scale):
    """Host-side O(N) prep: augmented (B,6,NX) / (B,6,NY) factor matrices."""
    s = np.clip(scale.astype(np.float64), 1e-6, 1e6)
    sx = np.clip(sample_x.astype(np.float64) @ s, 1e-10, 1e6)  # (B,NX)
    sy = np.clip(sample_y.astype(np.float64) @ s, 1e-10, 1e6)  # (B,NY)
    u = np.sqrt(sx)
    v = np.sqrt(sy)
    x64 = x.astype(np.float64)
    y64 = y.astype(np.float64)
    sqx = (x64 * x64).sum(-1)
    sqy = (y64 * y64).sum(-1)
    one_x = np.ones_like(u)
    XA = np.stack(
        [
            -2.0 * x64[..., 0] / u,
            -2.0 * x64[..., 1] / u,
            -2.0 * x64[..., 2] / u,
            sqx / u,
            1.0 / u,
            one_x,
        ],
        axis=1,
    )  # (B, 6, NX)
    YA = np.stack(
        [
            y64[..., 0] / v,
            y64[..., 1] / v,
            y64[..., 2] / v,
            1.0 / v,
            sqy / v,
            np.ones_like(v),
        ],
        axis=1,
    )  # (B, 6, NY)
    return XA, YA


def _split3(a64):
    """float64 (B,6,L) -> three bf16 (B,6,L) planes: hi, mid, lo."""
    import ml_dtypes

    bf = ml_dtypes.bfloat16
    a32 = a64.astype(np.float32)
    h = a32.astype(bf)
    r1 = a32 - h.astype(np.float32)
    m = r1.astype(bf)
    r2 = r1 - m.astype(np.float32)
    l = r2.astype(bf)
    return h, m, l


def _pack_rows(x, y, sample_x, sample_y, scale):
    """Returns per-core packed (KR, R+NY) bf16 inputs."""
    XA, YA = _augment(x, y, sample_x, sample_y, scale)
    xh, xm, xl = _split3(XA)
    yh, ym, yl = _split3(YA)
    # 6 cross-term pairs capturing (hi+mid+lo)x(hi+mid+lo) down to 2^-24
    XROWS = np.concatenate([xh, xh, xm, xh, xm, xl], axis=1)  # (B, 36, NX)
    YROWS = np.concatenate([yh, ym, yh, yl, ym, yh], axis=1)  # (B, 36, NY)
    ins = []
    for c in range(NCORES):
        b, half = divmod(c, NCORES // B)
        xa_c = XROWS[b][:, half * R : (half + 1) * R]
        ins.append(np.ascontiguousarray(np.concatenate([xa_c, YROWS[b]], axis=1)))
    return ins


def _run(inputs, trace=False):
    from concourse.bass_utils import run_bass_kernel_spmd

    ins = _pack_rows(
        inputs["x"], inputs["y"], inputs["sample_x"], inputs["sample_y"], inputs["scale"]
    )
    nc = _get_program()
    in_maps = [{"xya": a} for a in ins]
    res = run_bass_kernel_spmd(nc, in_maps, list(range(NCORES)), trace=trace)
    out = np.empty((B, NX, NY), dtype=np.float32)
    for c in range(NCORES):
        b, half = divmod(c, NCORES // B)
        out[b, half * R : (half + 1) * R, :] = res.results[c]["out"]
    return out, res


def kernel(x, y, sample_x, sample_y, scale):
    out, _ = _run(
        {
            "x": np.asarray(x),
            "y": np.asarray(y),
            "sample_x": np.asarray(sample_x),
            "sample_y": np.asarray(sample_y),
            "scale": np.asarray(scale),
        }
    )
    return out


# revision 22
# speedup vs baseline: 1.0077x; 1.0077x over previous
"""TRN2 Bass kernel for nn_BatchedCauchyKernel3d.

reference:
    d   = clip(||x_n||^2 + ||y_m||^2 - 2 x_n.y_m, 1e-10, 1e6)
    sxy = sqrt(clip(scale_x_n * scale_y_m, 1e-10, 1e12))
    out = 1 / (1 + d / sxy)

Rewrite: with u_n = sqrt(scale_x_n), v_m = sqrt(scale_y_m):
    1 + d/sxy = sum_k XA[k,n] * YA[k,m]      (K = 6 augmented contraction)
      XA = [-2 x1/u, -2 x2/u, -2 x3/u, ||x||^2/u, 1/u, 1]
      YA = [   y1/v,    y2/v,    y3/v,       1/v, ||y||^2/v, 1]
so the whole kernel matrix is ONE matmul followed by an elementwise
reciprocal.  The matmul runs in bf16 with a 3-way hi/mid/lo split of each
operand (6 cross-term pairs -> K = 36), which reproduces fp32 accuracy at
full (1 col/cycle) PE speed; fp32-native matmuls are 4x slower on TRN2.

Sharding: 8 cores, core c owns batch c//2, row half c%2 -> a (2048, 4096)
f32 output block per core (the output DMA of 32 MB/core is the roofline).
"""

import sys

if "/opt/trn_rl_repo" not in sys.path:
    sys.path.insert(0, "/opt/trn_rl_repo")

import numpy as np

B, NX, NY, FDIM = 4, 4096, 4096, 16
NCORES = 8
R = B * NX // NCORES  # 2048 rows per core
KPAIRS = 6  # (h,h),(h,m),(m,h),(h,l),(m,m),(l,h)
KR = 6 * KPAIRS  # 36

_CACHE = {}


def _build_program(rows, ny):
    from contextlib import ExitStack

    import concourse.tile as tile
    from concourse import bacc, mybir

    BF16 = mybir.dt.bfloat16
    F32 = mybir.dt.float32

    NB = 512  # matmul moving free dim (one PSUM bank of fp32)
    CH = 2048  # reciprocal chunk = 4 PSUM banks

    nc = bacc.Bacc("TRN2", target_bir_lowering=False, debug=False)
    xya = nc.declare_dram_parameter("xya", [KR, rows + ny], BF16, isOutput=False)
    out = nc.declare_dram_parameter("out", [rows, ny], F32, isOutput=True)

    with ExitStack() as ctx:
        tc = ctx.enter_context(tile.TileContext(nc))
        const = ctx.enter_context(tc.tile_pool(name="const", bufs=1))
        psum = ctx.enter_context(tc.tile_pool(name="psum", bufs=2, space="PSUM"))
        outp = ctx.enter_context(tc.tile_pool(name="outp", bufs=8))

        # Load the 36 contraction rows (split by column range across three
        # engines' DMA queues so the first matmuls only wait on the slices
        # they read), then duplicate them on-chip to partitions 64-99 so
        # matmuls can alternate PE row-groups and run concurrently in
        # disjoint quadrants of the array.
        xya_sb = const.tile([64 + KR, rows + ny], BF16)
        ranges = [(0, rows + NB), (rows + NB, rows + CH), (rows + CH, rows + ny)]
        engines = [nc.scalar, nc.sync, nc.gpsimd]
        for (lo, hi), eng in zip(ranges, engines):
            eng.dma_start(xya_sb[0:KR, lo:hi], xya[:, lo:hi])
            eng.dma_start(xya_sb[64 : 64 + KR, lo:hi], xya_sb[0:KR, lo:hi])

        for m in range(rows // 128):
            for h in range(ny // CH):
                ps = psum.tile([128, CH], F32, tag="ps")
                for j in range(CH // NB):
                    col = h * CH + j * NB
                    # first row-tile stays on group A: its matmuls gate the
                    # pipeline ramp and must not wait for the duplicate copy
                    g = 0 if m == 0 else 64 * (j % 2)
                    nc.tensor.matmul(
                        ps[:, j * NB : (j + 1) * NB],
                        xya_sb[g : g + KR, m * 128 : (m + 1) * 128],
                        xya_sb[g : g + KR, rows + col : rows + col + NB],
                        start=True,
                        stop=True,
                        tile_position=(g, 0),
                    )
                # fine-grained epilogue for the first row-tile so output DMA
                # starts as early as possible
                ot = outp.tile([128, CH], F32)
                if m == 0:
                    for j in range(CH // NB):
                        sl = slice(j * NB, (j + 1) * NB)
                        nc.vector.reciprocal_approx_fast(out=ot[:, sl], in_=ps[:, sl])
                        nc.sync.dma_start(
                            out[0:128, h * CH + j * NB : h * CH + (j + 1) * NB],
                            ot[:, sl],
                        )
                else:
                    nc.vector.reciprocal_approx_fast(out=ot[:], in_=ps[:])
                    nc.sync.dma_start(
                        out[m * 128 : (m + 1) * 128, h * CH : (h + 1) * CH], ot[:]
                    )

    nc.compile()
    return nc


def _get_program(rows=R, ny=NY):
    key = (rows, ny)
    if key not in _CACHE:
        _CACHE[key] = _build_program(rows, ny)
    return _CACHE[key]


def _augment(x, y, sample_x, sample_y, scale):
    """Host-side O(N) prep: augmented (B,6,NX) / (B,6,NY) factor matrices."""
    s = np.clip(scale.astype(np.float64), 1e-6, 1e6)
    sx = np.clip(sample_x.astype(np.float64) @ s, 1e-10, 1e6)  # (B,NX)
    sy = np.clip(sample_y.astype(np.float64) @ s, 1e-10, 1e6)  # (B,NY)
    u = np.sqrt(sx)
    v = np.sqrt(sy)
    x64 = x.astype(np.float64)
    y64 = y.astype(np.float64)
    sqx = (x64 * x64).sum(-1)
    sqy = (y64 * y64).sum(-1)
    one_x = np.ones_like(u)
    XA = np.stack(
        [
            -2.0 * x64[..., 0] / u,
            -2.0 * x64[..., 1] / u,
            -2.0 * x64[..., 2] / u,
            sqx / u,
            1.0 / u,
            one_x,
        ],
        axis=1,
    )  # (B, 6, NX)
    YA = np.stack(
        [
            y64[..., 0] / v,
            y64[..., 1] / v,
            y64[..., 2] / v,
            1.0 / v,
            sqy / v,
            np.ones_like(v),
        ],
        axis=1,
    )  # (B, 6, NY)
    return XA, YA


def _split3(a64):
    """float64 (B,6,L) -> three bf16 (B,6,L) planes: hi, mid, lo."""
    import ml_dtypes

    bf = ml_dtypes.bfloat16
    a32 = a64.astype(np.float32)
    h = a32.astype(bf)
    r1 = a32 - h.astype(np.float32)
    m = r1.astype(bf)
    r2 = r1 - m.astype(np.float32)
    l = r2.astype(bf)
    return h, m, l


def _pack_rows(x, y, sample_x, sample_y, scale):
    """Returns per-core packed (KR, R+NY) bf16 inputs."""
    XA, YA = _augment(x, y, sample_x, sample_y, scale)
    xh, xm, xl = _split3(XA)
    yh, ym, yl = _split3(YA)
    # 6 cross-term pairs capturing (hi+mid+lo)x(hi+mid+lo) down to 2^-24
    XROWS = np.concatenate([xh, xh, xm, xh, xm, xl], axis=1)  # (B, 36, NX)
    YROWS = np.concatenate([yh, ym, yh, yl, ym, yh], axis=1)  # (B, 36, NY)
    ins = []
    for c in range(NCORES):
        b, half = divmod(c, NCORES // B)
        xa_c = XROWS[b][:, half * R : (half + 1) * R]
        ins.append(np.ascontiguousarray(np.concatenate([xa_c, YROWS[b]], axis=1)))
    return ins


def _run(inputs, trace=False):
    from concourse.bass_utils import run_bass_kernel_spmd

    ins = _pack_rows(
        inputs["x"], inputs["y"], inputs["sample_x"], inputs["sample_y"], inputs["scale"]
    )
    nc = _get_program()
    in_maps = [{"xya": a} for a in ins]
    res = run_bass_kernel_spmd(nc, in_maps, list(range(NCORES)), trace=trace)
    out = np.empty((B, NX, NY), dtype=np.float32)
    for c in range(NCORES):
        b, half = divmod(c, NCORES // B)
        out[b, half * R : (half + 1) * R, :] = res.results[c]["out"]
    return out, res


def kernel(x, y, sample_x, sample_y, scale):
    out, _ = _run(
        {
            "x": np.asarray(x),
            "y": np.asarray(y),
            "sample_x": np.asarray(sample_x),
            "sample_y": np.asarray(sample_y),
            "scale": np.asarray(scale),
        }
    )
    return out


# revision 25
# speedup vs baseline: 1.0148x; 1.0071x over previous
"""TRN2 Bass kernel for nn_BatchedCauchyKernel3d.

reference:
    d   = clip(||x_n||^2 + ||y_m||^2 - 2 x_n.y_m, 1e-10, 1e6)
    sxy = sqrt(clip(scale_x_n * scale_y_m, 1e-10, 1e12))
    out = 1 / (1 + d / sxy)

Rewrite: with u_n = sqrt(scale_x_n), v_m = sqrt(scale_y_m):
    1 + d/sxy = sum_k XA[k,n] * YA[k,m]      (K = 6 augmented contraction)
      XA = [-2 x1/u, -2 x2/u, -2 x3/u, ||x||^2/u, 1/u, 1]
      YA = [   y1/v,    y2/v,    y3/v,       1/v, ||y||^2/v, 1]
so the whole kernel matrix is ONE matmul followed by an elementwise
reciprocal.  The matmul runs in bf16 with a 3-way hi/mid/lo split of each
operand (6 cross-term pairs -> K = 36), which reproduces fp32 accuracy at
full (1 col/cycle) PE speed; fp32-native matmuls are 4x slower on TRN2.

Sharding: 8 cores, core c owns batch c//2, row half c%2 -> a (2048, 4096)
f32 output block per core (the output DMA of 32 MB/core is the roofline).
"""

import sys

if "/opt/trn_rl_repo" not in sys.path:
    sys.path.insert(0, "/opt/trn_rl_repo")

import numpy as np

B, NX, NY, FDIM = 4, 4096, 4096, 16
NCORES = 8
R = B * NX // NCORES  # 2048 rows per core
KPAIRS = 6  # (h,h),(h,m),(m,h),(h,l),(m,m),(l,h)
KR = 6 * KPAIRS  # 36

_CACHE = {}


def _build_program(rows, ny):
    from contextlib import ExitStack

    import concourse.tile as tile
    from concourse import bacc, mybir

    BF16 = mybir.dt.bfloat16
    F32 = mybir.dt.float32

    NB = 512  # matmul moving free dim (one PSUM bank of fp32)
    CH = 2048  # reciprocal chunk = 4 PSUM banks

    nc = bacc.Bacc("TRN2", target_bir_lowering=False, debug=False)
    xya = nc.declare_dram_parameter("xya", [KR, rows + ny], BF16, isOutput=False)
    out = nc.declare_dram_parameter("out", [rows, ny], F32, isOutput=True)

    with ExitStack() as ctx:
        tc = ctx.enter_context(tile.TileContext(nc))
        const = ctx.enter_context(tc.tile_pool(name="const", bufs=1))
        psum = ctx.enter_context(tc.tile_pool(name="psum", bufs=2, space="PSUM"))
        outp = ctx.enter_context(tc.tile_pool(name="outp", bufs=4))
        outpm = ctx.enter_context(tc.tile_pool(name="outpm", bufs=3))

        # Load the 36 contraction rows (split by column range across three
        # engines' DMA queues so the first matmuls only wait on the slices
        # they read), then duplicate them on-chip to partitions 64-99 so
        # matmuls can alternate PE row-groups and run concurrently in
        # disjoint quadrants of the array.
        xya_sb = const.tile([64 + KR, rows + ny], BF16)
        ranges = [(0, rows + NB), (rows + NB, rows + CH), (rows + CH, rows + ny)]
        engines = [nc.scalar, nc.sync, nc.gpsimd]
        for (lo, hi), eng in zip(ranges, engines):
            eng.dma_start(xya_sb[0:KR, lo:hi], xya[:, lo:hi])
            eng.dma_start(xya_sb[64 : 64 + KR, lo:hi], xya_sb[0:KR, lo:hi])

        for m in range(rows // 128):
            otm = None if m == 0 else outpm.tile([128, ny], F32, tag="otm")
            for h in range(ny // CH):
                ps = psum.tile([128, CH], F32, tag="ps")
                for j in range(CH // NB):
                    col = h * CH + j * NB
                    # first row-tile stays on group A: its matmuls gate the
                    # pipeline ramp and must not wait for the duplicate copy
                    g = 0 if m == 0 else 64 * (j % 2)
                    nc.tensor.matmul(
                        ps[:, j * NB : (j + 1) * NB],
                        xya_sb[g : g + KR, m * 128 : (m + 1) * 128],
                        xya_sb[g : g + KR, rows + col : rows + col + NB],
                        start=True,
                        stop=True,
                        tile_position=(g, 0),
                    )
                # fine-grained epilogue for the first row-tile so output DMA
                # starts as early as possible; whole-row 2MB DMAs afterwards
                if m == 0:
                    ot = outp.tile([128, CH], F32)
                    for j in range(CH // NB):
                        sl = slice(j * NB, (j + 1) * NB)
                        nc.vector.reciprocal_approx_fast(out=ot[:, sl], in_=ps[:, sl])
                        nc.sync.dma_start(
                            out[0:128, h * CH + j * NB : h * CH + (j + 1) * NB],
                            ot[:, sl],
                        )
                else:
                    nc.vector.reciprocal_approx_fast(
                        out=otm[:, h * CH : (h + 1) * CH], in_=ps[:]
                    )
            if m > 0:
                nc.sync.dma_start(out[m * 128 : (m + 1) * 128, :], otm[:])

    nc.compile()
    return nc


def _get_program(rows=R, ny=NY):
    key = (rows, ny)
    if key not in _CACHE:
        _CACHE[key] = _build_program(rows, ny)
    return _CACHE[key]


def _augment(x, y, sample_x, sample_y, scale):
    """Host-side O(N) prep: augmented (B,6,NX) / (B,6,NY) factor matrices."""
    s = np.clip(scale.astype(np.float64), 1e-6, 1e6)
    sx = np.clip(sample_x.astype(np.float64) @ s, 1e-10, 1e6)  # (B,NX)
    sy = np.clip(sample_y.astype(np.float64) @ s, 1e-10, 1e6)  # (B,NY)
    u = np.sqrt(sx)
    v = np.sqrt(sy)
    x64 = x.astype(np.float64)
    y64 = y.astype(np.float64)
    sqx = (x64 * x64).sum(-1)
    sqy = (y64 * y64).sum(-1)
    one_x = np.ones_like(u)
    XA = np.stack(
        [
            -2.0 * x64[..., 0] / u,
            -2.0 * x64[..., 1] / u,
            -2.0 * x64[..., 2] / u,
            sqx / u,
            1.0 / u,
            one_x,
        ],
        axis=1,
    )  # (B, 6, NX)
    YA = np.stack(
        [
            y64[..., 0] / v,
            y64[..., 1] / v,
            y64[..., 2] / v,
            1.0 / v,
            sqy / v,
            np.ones_like(v),
        ],
        axis=1,
    )  # (B, 6, NY)
    return XA, YA


def _split3(a64):
    """float64 (B,6,L) -> three bf16 (B,6,L) planes: hi, mid, lo."""
    import ml_dtypes

    bf = ml_dtypes.bfloat16
    a32 = a64.astype(np.float32)
    h = a32.astype(bf)
    r1 = a32 - h.astype(np.float32)
    m = r1.astype(bf)
    r2 = r1 - m.astype(np.float32)
    l = r2.astype(bf)
    return h, m, l


def _pack_rows(x, y, sample_x, sample_y, scale):
    """Returns per-core packed (KR, R+NY) bf16 inputs."""
    XA, YA = _augment(x, y, sample_x, sample_y, scale)
    xh, xm, xl = _split3(XA)
    yh, ym, yl = _split3(YA)
    # 6 cross-term pairs capturing (hi+mid+lo)x(hi+mid+lo) down to 2^-24
    XROWS = np.concatenate([xh, xh, xm, xh, xm, xl], axis=1)  # (B, 36, NX)
    YROWS = np.concatenate([yh, ym, yh, yl, ym, yh], axis=1)  # (B, 36, NY)
    ins = []
    for c in range(NCORES):
        b, half = divmod(c, NCORES // B)
        xa_c = XROWS[b][:, half * R : (half + 1) * R]
        ins.append(np.ascontiguousarray(np.concatenate([xa_c, YROWS[b]], axis=1)))
    return ins


def _run(inputs, trace=False):
    from concourse.bass_utils import run_bass_kernel_spmd

    ins = _pack_rows(
        inputs["x"], inputs["y"], inputs["sample_x"], inputs["sample_y"], inputs["scale"]
    )
    nc = _get_program()
    in_maps = [{"xya": a} for a in ins]
    res = run_bass_kernel_spmd(nc, in_maps, list(range(NCORES)), trace=trace)
    out = np.empty((B, NX, NY), dtype=np.float32)
    for c in range(NCORES):
        b, half = divmod(c, NCORES // B)
        out[b, half * R : (half + 1) * R, :] = res.results[c]["out"]
    return out, res


def kernel(x, y, sample_x, sample_y, scale):
    out, _ = _run(
        {
            "x": np.asarray(x),
            "y": np.asarray(y),
            "sample_x": np.asarray(sample_x),
            "sample_y": np.asarray(sample_y),
            "scale": np.asarray(scale),
        }
    )
    return out


# revision 28
# speedup vs baseline: 1.0669x; 1.0513x over previous
"""TRN2 Bass kernel for nn_BatchedCauchyKernel3d.

reference:
    d   = clip(||x_n||^2 + ||y_m||^2 - 2 x_n.y_m, 1e-10, 1e6)
    sxy = sqrt(clip(scale_x_n * scale_y_m, 1e-10, 1e12))
    out = 1 / (1 + d / sxy)

Rewrite: with u_n = sqrt(scale_x_n), v_m = sqrt(scale_y_m):
    1 + d/sxy = sum_k XA[k,n] * YA[k,m]      (K = 6 augmented contraction)
      XA = [-2 x1/u, -2 x2/u, -2 x3/u, ||x||^2/u, 1/u, 1]
      YA = [   y1/v,    y2/v,    y3/v,       1/v, ||y||^2/v, 1]
so the whole kernel matrix is ONE matmul followed by an elementwise
reciprocal.  The matmul runs in bf16 with a 3-way hi/mid/lo split of each
operand (6 cross-term pairs -> K = 36), which reproduces fp32 accuracy at
full (1 col/cycle) PE speed; fp32-native matmuls are 4x slower on TRN2.

Sharding: 8 cores, core c owns batch c//2, row half c%2 -> a (2048, 4096)
f32 output block per core (the output DMA of 32 MB/core is the roofline).
"""

import sys

if "/opt/trn_rl_repo" not in sys.path:
    sys.path.insert(0, "/opt/trn_rl_repo")

import numpy as np

B, NX, NY, FDIM = 4, 4096, 4096, 16
NCORES = 8
R = B * NX // NCORES  # 2048 rows per core
KPAIRS = 6  # (h,h),(h,m),(m,h),(h,l),(m,m),(l,h)
KR = 6 * KPAIRS  # 36

_CACHE = {}


def _build_program(rows, ny):
    from contextlib import ExitStack

    import concourse.tile as tile
    from concourse import bacc, mybir

    BF16 = mybir.dt.bfloat16
    F32 = mybir.dt.float32

    NB = 512  # matmul moving free dim (one PSUM bank of fp32)
    CH = 2048  # reciprocal chunk = 4 PSUM banks

    nc = bacc.Bacc("TRN2", target_bir_lowering=False, debug=False)
    xya = nc.declare_dram_parameter("xya", [KR, rows + ny], BF16, isOutput=False)
    out = nc.declare_dram_parameter("out", [rows, ny], F32, isOutput=True)

    with ExitStack() as ctx:
        tc = ctx.enter_context(tile.TileContext(nc))
        const = ctx.enter_context(tc.tile_pool(name="const", bufs=1))
        psum = ctx.enter_context(tc.tile_pool(name="psum", bufs=2, space="PSUM"))
        outp = ctx.enter_context(tc.tile_pool(name="outp", bufs=6))

        # Load the 36 contraction rows (split by column range across three
        # engines' DMA queues so the first matmuls only wait on the slices
        # they read), then duplicate them on-chip to partitions 64-99 so
        # matmuls can alternate PE row-groups and run concurrently in
        # disjoint quadrants of the array.
        xya_sb = const.tile([64 + KR, rows + ny], BF16)
        ranges = [(0, rows + NB), (rows + NB, rows + CH), (rows + CH, rows + ny)]
        engines = [nc.scalar, nc.sync, nc.gpsimd]
        for (lo, hi), eng in zip(ranges, engines):
            eng.dma_start(xya_sb[0:KR, lo:hi], xya[:, lo:hi])
            eng.dma_start(xya_sb[64 : 64 + KR, lo:hi], xya_sb[0:KR, lo:hi])

        for m in range(rows // 128):
            for h in range(ny // CH):
                ps = psum.tile([128, CH], F32, tag="ps")
                for j in range(CH // NB):
                    col = h * CH + j * NB
                    # first row-tile stays on group A: its matmuls gate the
                    # pipeline ramp and must not wait for the duplicate copy
                    g = 0 if m == 0 else 64 * (j % 2)
                    nc.tensor.matmul(
                        ps[:, j * NB : (j + 1) * NB],
                        xya_sb[g : g + KR, m * 128 : (m + 1) * 128],
                        xya_sb[g : g + KR, rows + col : rows + col + NB],
                        start=True,
                        stop=True,
                        tile_position=(g, 0),
                    )
                # fine-grained epilogue for the first row-tile so output DMA
                # starts as early as possible
                ot = outp.tile([128, CH], F32)
                if m == 0:
                    for j in range(CH // NB):
                        sl = slice(j * NB, (j + 1) * NB)
                        nc.vector.reciprocal_approx_fast(out=ot[:, sl], in_=ps[:, sl])
                        nc.sync.dma_start(
                            out[0:128, h * CH + j * NB : h * CH + (j + 1) * NB],
                            ot[:, sl],
                        )
                else:
                    nc.vector.reciprocal_approx_fast(out=ot[:], in_=ps[:])
                    nc.sync.dma_start(
                        out[m * 128 : (m + 1) * 128, h * CH : (h + 1) * CH], ot[:]
                    )

    nc.compile()
    return nc


def _get_program(rows=R, ny=NY):
    key = (rows, ny)
    if key not in _CACHE:
        _CACHE[key] = _build_program(rows, ny)
    return _CACHE[key]


def _augment(x, y, sample_x, sample_y, scale):
    """Host-side O(N) prep: augmented (B,6,NX) / (B,6,NY) factor matrices."""
    s = np.clip(scale.astype(np.float64), 1e-6, 1e6)
    sx = np.clip(sample_x.astype(np.float64) @ s, 1e-10, 1e6)  # (B,NX)
    sy = np.clip(sample_y.astype(np.float64) @ s, 1e-10, 1e6)  # (B,NY)
    u = np.sqrt(sx)
    v = np.sqrt(sy)
    x64 = x.astype(np.float64)
    y64 = y.astype(np.float64)
    sqx = (x64 * x64).sum(-1)
    sqy = (y64 * y64).sum(-1)
    one_x = np.ones_like(u)
    XA = np.stack(
        [
            -2.0 * x64[..., 0] / u,
            -2.0 * x64[..., 1] / u,
            -2.0 * x64[..., 2] / u,
            sqx / u,
            1.0 / u,
            one_x,
        ],
        axis=1,
    )  # (B, 6, NX)
    YA = np.stack(
        [
            y64[..., 0] / v,
            y64[..., 1] / v,
            y64[..., 2] / v,
            1.0 / v,
            sqy / v,
            np.ones_like(v),
        ],
        axis=1,
    )  # (B, 6, NY)
    return XA, YA


def _split3(a64):
    """float64 (B,6,L) -> three bf16 (B,6,L) planes: hi, mid, lo."""
    import ml_dtypes

    bf = ml_dtypes.bfloat16
    a32 = a64.astype(np.float32)
    h = a32.astype(bf)
    r1 = a32 - h.astype(np.float32)
    m = r1.astype(bf)
    r2 = r1 - m.astype(np.float32)
    l = r2.astype(bf)
    return h, m, l


def _pack_rows(x, y, sample_x, sample_y, scale):
    """Returns per-core packed (KR, R+NY) bf16 inputs."""
    XA, YA = _augment(x, y, sample_x, sample_y, scale)
    xh, xm, xl = _split3(XA)
    yh, ym, yl = _split3(YA)
    # 6 cross-term pairs capturing (hi+mid+lo)x(hi+mid+lo) down to 2^-24
    XROWS = np.concatenate([xh, xh, xm, xh, xm, xl], axis=1)  # (B, 36, NX)
    YROWS = np.concatenate([yh, ym, yh, yl, ym, yh], axis=1)  # (B, 36, NY)
    ins = []
    for c in range(NCORES):
        b, half = divmod(c, NCORES // B)
        xa_c = XROWS[b][:, half * R : (half + 1) * R]
        ins.append(np.ascontiguousarray(np.concatenate([xa_c, YROWS[b]], axis=1)))
    return ins


def _run(inputs, trace=False):
    from concourse.bass_utils import run_bass_kernel_spmd

    ins = _pack_rows(
        inputs["x"], inputs["y"], inputs["sample_x"], inputs["sample_y"], inputs["scale"]
    )
    nc = _get_program()
    in_maps = [{"xya": a} for a in ins]
    res = run_bass_kernel_spmd(nc, in_maps, list(range(NCORES)), trace=trace)
    out = np.empty((B, NX, NY), dtype=np.float32)
    for c in range(NCORES):
        b, half = divmod(c, NCORES // B)
        out[b, half * R : (half + 1) * R, :] = res.results[c]["out"]
    return out, res


def kernel(x, y, sample_x, sample_y, scale):
    out, _ = _run(
        {
            "x": np.asarray(x),
            "y": np.asarray(y),
            "sample_x": np.asarray(sample_x),
            "sample_y": np.asarray(sample_y),
            "scale": np.asarray(scale),
        }
    )
    return out
